# revision 22
# baseline (speedup 1.0000x reference)
# Trainium2 Bass kernel for the AdAP_PZ loss function.
#
# Math notes (why this is O(N), not O(N^2)):
#   sur[i,j] = max(1 - (f_i - f_j), 0)^2 with f in [0,1), so the hinge is
#   never active and sur[i,j] = ((1-f_i) + f_j)^2 exactly. Row sums are then
#   closed-form in global moments of f:
#     S_i  = sum_j sur[i,j]          = N*a_i^2      + 2*a_i*sum(f)   + sum(f^2)
#     SP_i = sum_j sur[i,j]*pos_j    = npos*a_i^2   + 2*a_i*sum(tf)  + sum(tf^2)
#   with a_i = 1 - f_i. The p*sur contraction reduces row-wise:
#     sum_j p[i,j]*sur[i,j] = (up_new_i*S_i - ua_new_i*SP_i) / ua_new_i^2
#   on positive rows (negative rows are masked by t_i), and expanding
#   ua_new = 0.9*ua + (g/N)*S, up_new = 0.9*up + (g/N)*SP gives
#     up_new*S - ua_new*SP = 0.9*(up*S - ua*SP)
#   exactly. We compute with S' = (GAMMA/N)*S and SP' = (GAMMA/N)*SP and fold
#   the 0.9 and 1/GAMMA into the final scalar combine. The adversarial KL
#   splits as sum([f|a]*[ln f|ln a]) - sum([f|a]*[ln q|ln qc]) so both halves
#   are independent fused multiply+row-sum ops, subtracted as scalars at the
#   end. The ones matrix used for partition reduction carries a 1/N factor,
#   so both reduction matmuls produce means and no explicit /N op is needed.
#
# Distribution: the whole computation is ~50K elements of vector work, far
# below any useful sharding granularity, so each of the 8 cores runs the
# identical replicated kernel (no collectives, no deadlock surface) and the
# host reads core 0's scalar.
#
# Hardware/schedule notes:
#   - tensor_tensor_reduce, gpsimd accum_out, and AluOp.divide fail on this
#     runtime; DVE scalar_tensor_tensor+accum_out is the working fused
#     row-sum form.
#   - The ACT "natural_log" function set contains ln AND identity, so a dummy
#     Ln issued before the data arrives preloads the table (~1.3us) and the
#     identity ops (a = 1-f, qc = 1-q) plus the packed Ln run with no reload.
#   - All four logarithms are ONE activation over the packed [f|a|q|qc] tile.
#   - DVE and Pool each run a forced stall-free instruction order
#     (sync=False scheduling edges). Pool carries the SP' chain and the
#     [up|ua]*[S'|SP'] product while the DVE runs the S' chain, the
#     reciprocal path, and the fused KL row-sums in the gap.

import numpy as np

P = 128        # SBUF partitions
F = 96         # free-dim columns; P*F == N
N = 12288
GAMMA = 0.1
NCORES = 8

_NC_CACHE = None


def _build_nc():
    from contextlib import ExitStack

    import concourse.bacc as bacc
    import concourse.mybir as mybir
    import concourse.tile as tile
    from concourse.tile_rust import add_dep_helper

    dt = mybir.dt.float32
    Act = mybir.ActivationFunctionType
    Alu = mybir.AluOpType
    Ax = mybir.AxisListType

    nc = bacc.Bacc(
        "TRN2",
        target_bir_lowering=False,
        debug=False,
        enable_asserts=False,
        num_devices=NCORES,
    )
    # Packed input: columns [f | t | up | ua | q], each P x F.
    inp = nc.dram_tensor("inp", [P, 5 * F], dt, kind="ExternalInput")
    out = nc.dram_tensor("out", [1, 1], dt, kind="ExternalOutput")

    dve_chain = []   # forced DVE order
    pool_chain = []  # forced Pool order

    def dve(inst):
        dve_chain.append(inst)
        return inst

    def plq(inst):
        pool_chain.append(inst)
        return inst

    with tile.TileContext(nc) as tc, ExitStack() as ctx:
        pool = ctx.enter_context(tc.tile_pool(name="sb", bufs=1))
        psum = ctx.enter_context(tc.tile_pool(name="ps", bufs=1, space="PSUM"))

        x = pool.tile([P, 4 * F], dt)   # [f | t | up | ua]
        L = pool.tile([P, 4 * F], dt)   # [f | a | q | qc] -> packed Ln input
        nc.sync.dma_start(x[:, 0 : 2 * F], inp.ap()[:, 0 : 2 * F])
        nc.sync.dma_start(L[:, 2 * F : 3 * F], inp.ap()[:, 4 * F : 5 * F])
        nc.sync.dma_start(x[:, 2 * F : 4 * F], inp.ap()[:, 2 * F : 4 * F])
        f = x[:, 0 * F : 1 * F]
        t = x[:, 1 * F : 2 * F]
        upua = x[:, 2 * F : 4 * F]
        qL = L[:, 2 * F : 3 * F]

        # Constants (built while the DMA is in flight).
        ones128 = pool.tile([P, P], dt)
        nc.gpsimd.memset(ones128[:], 1.0 / N)  # reduction matmuls give means
        consts = pool.tile([P, 2], dt)  # [1.0, 1e-12]
        dve(nc.vector.memset(consts[:, 0:1], 1.0))
        dve(nc.vector.memset(consts[:, 1:2], 1e-12))
        facA = pool.tile([P, 2], dt)    # [2*GAMMA, GAMMA] on mean moments
        dve(nc.vector.memset(facA[:, 0:1], 2 * GAMMA))
        dve(nc.vector.memset(facA[:, 1:2], GAMMA))
        facB = pool.tile([P, 3], dt)
        dve(nc.vector.memset(facB[:, 0:1], 2 * GAMMA))
        dve(nc.vector.memset(facB[:, 1:2], GAMMA))
        dve(nc.vector.memset(facB[:, 2:3], GAMMA))

        # Warm the ACT natural_log function set before the data arrives.
        warm = pool.tile([P, 1], dt)
        nc.scalar.activation(out=warm[:], in_=consts[:, 0:1], func=Act.Ln,
                             bias=consts[:, 1:2], scale=1.0)

        # Duplicate f into the Ln tile (GpSimd 1-input copy, off the DVE).
        plq(nc.gpsimd.tensor_copy(L[:, 0:F], f))
        # a = 1 - f and qc = 1 - q on ACT (identity is in the natural_log set)
        nc.scalar.activation(out=L[:, F : 2 * F], in_=f, func=Act.Identity,
                             bias=consts[:, 0:1], scale=-1.0)
        nc.scalar.activation(out=L[:, 3 * F : 4 * F], in_=qL, func=Act.Identity,
                             bias=consts[:, 0:1], scale=-1.0)
        a = L[:, F : 2 * F]
        # LL = ln(L + 1e-12) = [ln p | ln(1-p) | ln(q+eps) | ln(1-q+eps)]
        LL = pool.tile([P, 4 * F], dt)
        nc.scalar.activation(out=LL[:], in_=L[:], func=Act.Ln,
                             bias=consts[:, 1:2], scale=1.0)
        # After the Ln, overwrite the spent q|qc half of L with -f|-a so ONE
        # fused multiply/row-sum yields entropy-minus-cross directly.
        nc.scalar.activation(out=L[:, 2 * F : 4 * F], in_=L[:, 0 : 2 * F],
                             func=Act.Identity, bias=0.0, scale=-1.0)

        # ---- Phase 1 (DVE): global moments
        # r = [sum f, sum f^2, sum tf, sum tf^2, npos] (pre-/N via ones128)
        r = pool.tile([P, 5], dt)
        tf = pool.tile([P, F], dt)
        j1 = pool.tile([P, F], dt)
        j2 = pool.tile([P, F], dt)
        # One 3D reduce computes sum(f) and sum(t) together (strided out AP
        # writes columns 0 and 4 of r).
        dve(nc.vector.reduce_sum(
            out=r[:, 0:5:4],
            in_=x[:, 0 : 2 * F].rearrange("p (k f) -> p k f", k=2),
            axis=Ax.X))
        dve(nc.vector.scalar_tensor_tensor(out=j1[:], in0=f, scalar=1.0, in1=f,
                                           op0=Alu.mult, op1=Alu.mult,
                                           accum_out=r[:, 1:2]))
        dve(nc.vector.scalar_tensor_tensor(out=tf[:], in0=t, scalar=1.0, in1=f,
                                           op0=Alu.mult, op1=Alu.mult,
                                           accum_out=r[:, 2:3]))
        dve(nc.vector.scalar_tensor_tensor(out=j2[:], in0=tf[:], scalar=1.0,
                                           in1=f, op0=Alu.mult, op1=Alu.mult,
                                           accum_out=r[:, 3:4]))

        # Split partition-sum matmuls: A-columns unblock the S' chain early.
        RpA = psum.tile([P, 2], dt)
        nc.tensor.matmul(RpA[:], ones128[:], r[:, 0:2], start=True, stop=True)
        RpB = psum.tile([P, 3], dt)
        nc.tensor.matmul(RpB[:], ones128[:], r[:, 2:5], start=True, stop=True)
        CA = pool.tile([P, 2], dt)      # [cS1, cS2]
        dve(nc.vector.tensor_mul(CA[:], RpA[:], facA[:]))
        CB = pool.tile([P, 3], dt)      # [cP1, cP2, cP0]
        dve(nc.vector.tensor_mul(CB[:], RpB[:], facB[:]))

        # SPK = [S' | SP']
        SPK = pool.tile([P, 2 * F], dt)
        # ---- S' chain (DVE): S' = a*(GAMMA*a + cS1) + cS2
        Sterm = pool.tile([P, F], dt)
        Sp = pool.tile([P, F], dt)
        dve(nc.vector.tensor_scalar(out=Sterm[:], in0=a, scalar1=GAMMA,
                                    scalar2=CA[:, 0:1], op0=Alu.mult,
                                    op1=Alu.add))
        dve(nc.vector.tensor_mul(Sp[:], a, Sterm[:]))
        dve(nc.vector.tensor_scalar_add(SPK[:, 0:F], Sp[:], CA[:, 1:2]))
        # ---- SP' chain (Pool): SP' = a*(cP0*a + cP1) + cP2
        Pterm = pool.tile([P, F], dt)
        Pp = pool.tile([P, F], dt)
        plq(nc.gpsimd.tensor_scalar(out=Pterm[:], in0=a, scalar1=CB[:, 2:3],
                                    scalar2=CB[:, 0:1], op0=Alu.mult,
                                    op1=Alu.add))
        plq(nc.gpsimd.tensor_mul(Pp[:], a, Pterm[:]))
        plq(nc.gpsimd.tensor_scalar_add(SPK[:, F : 2 * F], Pp[:], CB[:, 1:2]))
        # m12 = [up*S' | ua*SP'] (Pool, runs while DVE does the rec path)
        m12 = pool.tile([P, 2 * F], dt)
        plq(nc.gpsimd.tensor_mul(m12[:], upua, SPK[:]))

        # uan = 0.9*ua + S'; den = uan^2; rec = 1/den  (DVE)
        uan = pool.tile([P, F], dt)
        dve(nc.vector.scalar_tensor_tensor(out=uan[:], in0=x[:, 3 * F : 4 * F],
                                           scalar=1.0 - GAMMA, in1=SPK[:, 0:F],
                                           op0=Alu.mult, op1=Alu.add))
        den = pool.tile([P, F], dt)
        dve(nc.vector.tensor_mul(den[:], uan[:], uan[:]))
        rec = pool.tile([P, F], dt)
        dve(nc.vector.reciprocal(rec[:], den[:]))

        # ---- Adversarial KL combine (fills the m12 wait): one fused
        # multiply/row-sum over [f|a|-f|-a] * LL = entropy minus cross.
        rr = pool.tile([P, 2], dt)  # [nat, adv]
        ej = pool.tile([P, 4 * F], dt)
        dve(nc.vector.scalar_tensor_tensor(out=ej[:], in0=L[:], scalar=1.0,
                                           in1=LL[:], op0=Alu.mult,
                                           op1=Alu.mult,
                                           accum_out=rr[:, 1:2]))

        # ---- nat join (DVE): num = m1 - m2; contrib = t*num*rec ----
        num = pool.tile([P, F], dt)
        dve(nc.vector.tensor_sub(num[:], m12[:, 0:F], m12[:, F : 2 * F]))
        pr = pool.tile([P, F], dt)
        dve(nc.vector.tensor_mul(pr[:], num[:], rec[:]))
        cj = pool.tile([P, F], dt)
        dve(nc.vector.scalar_tensor_tensor(out=cj[:], in0=pr[:], scalar=1.0,
                                           in1=t, op0=Alu.mult, op1=Alu.mult,
                                           accum_out=rr[:, 0:1]))
        # npos reciprocal for the final combine (fills the mm2 flight time)
        rnp = pool.tile([1, 1], dt)
        dve(nc.vector.reciprocal(rnp[:], RpB[0:1, 2:3]))  # = N/npos
        rnpg = pool.tile([1, 1], dt)
        dve(nc.vector.tensor_scalar_mul(rnpg[:], rnp[:], (1.0 - GAMMA) / GAMMA))

        # ---- Final: partition-mean rr, combine scalars ----
        # Fp = [nat_sum/N, adv_sum/N] broadcast to all partitions
        Fp = psum.tile([P, 2], dt)
        nc.tensor.matmul(Fp[:], ones128[:], rr[:], start=True, stop=True)
        v1 = pool.tile([1, 1], dt)
        dve(nc.vector.tensor_mul(v1[:], Fp[0:1, 0:1], rnpg[:]))
        res = pool.tile([1, 1], dt)
        dve(nc.vector.tensor_tensor(out=res[:], in0=Fp[0:1, 1:2], in1=v1[:],
                                    op=Alu.add))
        nc.sync.dma_start(out.ap(), res[:])

        for prev, nxt in zip(dve_chain, dve_chain[1:]):
            add_dep_helper(nxt.ins, prev.ins, sync=False,
                           reason="forced DVE stream order")
        for prev, nxt in zip(pool_chain, pool_chain[1:]):
            add_dep_helper(nxt.ins, prev.ins, sync=False,
                           reason="forced Pool stream order")

    nc.compile()
    return nc


def _get_nc():
    global _NC_CACHE
    if _NC_CACHE is None:
        _NC_CACHE = _build_nc()
    return _NC_CACHE


def _pack_inputs(y_pred, y_pred_adv, u_all, u_pos, y_true, index_s):
    f = np.asarray(y_pred, dtype=np.float32).reshape(-1)
    q = np.asarray(y_pred_adv, dtype=np.float32).reshape(-1)
    t = (np.asarray(y_true).reshape(-1) == 1).astype(np.float32)
    idx = np.asarray(index_s).reshape(-1).astype(np.int64)
    ua = np.asarray(u_all, dtype=np.float32).reshape(-1)[idx]
    up = np.asarray(u_pos, dtype=np.float32).reshape(-1)[idx]
    packed = np.stack([f, t, up, ua, q]).reshape(5, P, F).transpose(1, 0, 2)
    return np.ascontiguousarray(packed.reshape(P, 5 * F))


def kernel(y_pred, y_pred_adv, u_all, u_pos, y_true, index_s, _trace=False):
    import time

    from concourse.bass_utils import run_bass_kernel_spmd

    inp = _pack_inputs(y_pred, y_pred_adv, u_all, u_pos, y_true, index_s)
    nc = _get_nc()
    in_maps = [{"inp": inp} for _ in range(NCORES)]
    # The fleet occasionally reports a transient NRT_EXEC_UNIT_UNRECOVERABLE
    # left over from an earlier crashed process; retry a couple of times.
    last_exc = None
    for attempt in range(3):
        try:
            bres = run_bass_kernel_spmd(nc, in_maps,
                                        core_ids=list(range(NCORES)),
                                        trace=_trace)
            break
        except Exception as exc:  # noqa: BLE001
            last_exc = exc
            time.sleep(10 * (attempt + 1))
    else:
        raise last_exc
    val = np.asarray(bres.results[0]["out"], dtype=np.float32).reshape(())
    if _trace:
        return val, bres
    return val
